# revision 25
# baseline (speedup 1.0000x reference)
"""GCN (2-layer graph convolution, symmetric norm) on 8 TRN2 NeuronCores.

Node-sharded graph/data-parallel, 3 launches (per sharding hint):
 - Phase A (node-sharded MLP, transposed dataflow): each core computes
   h1s = (lrelu(x@W1+b1)@W2+b2) * rsqrt(max(deg_s,1)) for its 12500-node
   range (fp8e4m3 table out). Host supplies x pre-transposed, so zero
   on-device transposes:  h1T = W1^T @ xT (lhsT=W1), Lrelu+bias on the
   scalar engine, then h2 = h1T^T @ W2 (lhsT=h1T) lands row-major.
 - Halo exchange between launches is host-mediated: the host gathers
   h1s[senders] / h2s[senders] into per-core, receiver-sorted edge-row
   streams (partition-major layout), so each launch only does full-
   bandwidth sequential DMA - no on-device random access.
 - Phase B (edge-sharded by receiver): per 128-receiver block, segment-sum
   the streamed fp8 rows with one-hot matmuls accumulating the TRANSPOSED
   aggregate (lhsT=g, rhs=onehot), so the Lrelu'd result feeds the decode
   matmul directly as lhsT with no transpose. inv_r/bias/inv_s algebra is
   folded into a rank-1 bias matmul (sqrt(deg_r) x bd) and one per-
   partition scale (inv_r*inv_s). One-hot matrices for 49 blocks x ksub
   are built in ONE DVE op per supertile via 3D broadcast is_equal.
 - Phase C: same aggregation over bf16 h2s edge rows (40-wide), then a
   fused softmax: Exp activation with scale=inv_r, bias=-inv_r*max,
   accum_out for the denominator.
"""

import numpy as np
import ml_dtypes

N = 100000
E = 600000
D = 128
C = 40
NCORES = 8
NS = N // NCORES          # 12500 nodes per core
P = 128
NB = (NS + P - 1) // P    # 98 blocks per core
NPAD = NB * P             # 12544
SUPA = 14                 # phase-A supertile (blocks per DMA); NB % SUPA == 0
SUPB = 7                  # phase-B/C supertile (blocks per stream tile)
NSUP = NB // SUPB         # 14

BF16 = ml_dtypes.bfloat16
F8 = ml_dtypes.float8_e4m3


def _ctx():
    from concourse import bass, bacc, mybir, tile
    return bass, bacc, mybir, tile


def _loop(tc, repeat):
    import contextlib
    if repeat > 1:
        return tc.For_i(0, repeat)
    return contextlib.nullcontext()


def _build_phase_a(repeat=1):
    bass, bacc, mybir, tile = _ctx()
    f32 = mybir.dt.float32
    bf16 = mybir.dt.bfloat16
    f8 = mybir.dt.float8e4
    nc = bacc.Bacc("TRN2", target_bir_lowering=False, debug=False)
    xT_ext = nc.declare_dram_parameter("xT", [D, NPAD], bf16, isOutput=False)
    w1_ext = nc.declare_dram_parameter("W1", [D, D], bf16, isOutput=False)
    b1_ext = nc.declare_dram_parameter("b1c", [D, 1], f32, isOutput=False)
    w2_ext = nc.declare_dram_parameter("W2", [D, D], bf16, isOutput=False)
    b2_ext = nc.declare_dram_parameter("b2r", [1, D], bf16, isOutput=False)
    ones_ext = nc.declare_dram_parameter("ones_row", [1, P], bf16, isOutput=False)
    invs_ext = nc.declare_dram_parameter("invs_pc", [P, NB], f32, isOutput=False)
    out_ext = nc.declare_dram_parameter("h1s_pm", [P, NB, D], f8, isOutput=True)

    with tile.TileContext(nc) as tc:
        with (
            tc.tile_pool(name="const", bufs=1) as cp,
            tc.tile_pool(name="xin", bufs=3) as xp,
            tc.tile_pool(name="mid", bufs=6) as sp,
            tc.tile_pool(name="outp", bufs=3) as op,
            tc.tile_pool(name="psum1", bufs=3, space="PSUM") as pp1,
            tc.tile_pool(name="psum2", bufs=3, space="PSUM") as pp2,
        ):
            w1 = cp.tile([D, D], dtype=bf16)
            nc.sync.dma_start(out=w1[:], in_=w1_ext[:])
            b1c = cp.tile([D, 1], dtype=f32)
            nc.sync.dma_start(out=b1c[:], in_=b1_ext[:])
            w2 = cp.tile([D, D], dtype=bf16)
            nc.sync.dma_start(out=w2[:], in_=w2_ext[:])
            b2r = cp.tile([1, D], dtype=bf16)
            nc.sync.dma_start(out=b2r[:], in_=b2_ext[:])
            ones = cp.tile([1, P], dtype=bf16)
            nc.sync.dma_start(out=ones[:], in_=ones_ext[:])
            invs = cp.tile([P, NB], dtype=f32)
            nc.sync.dma_start(out=invs[:], in_=invs_ext[:])

            with _loop(tc, repeat):
                for s in range(NB // SUPA):
                    xt = xp.tile([D, SUPA * P], dtype=bf16)
                    nc.sync.dma_start(
                        out=xt[:], in_=xT_ext[:, s * SUPA * P:(s + 1) * SUPA * P]
                    )
                    o_super = op.tile([P, SUPA, D], dtype=f8)
                    # mm1 + lrelu batched over groups of 4 blocks (one PSUM bank)
                    h1Ts = []
                    for g0 in range(0, SUPA, 4):
                        gw = min(4, SUPA - g0)
                        ps1 = pp1.tile([D, 4 * P], dtype=f32, space="PSUM")
                        nc.tensor.matmul(
                            out=ps1[:, :gw * P], lhsT=w1[:],
                            rhs=xt[:, g0 * P:(g0 + gw) * P],
                            start=True, stop=True,
                        )
                        h1T = sp.tile([D, 4 * P], dtype=bf16)
                        nc.scalar.activation(
                            out=h1T[:, :gw * P], in_=ps1[:, :gw * P],
                            func=mybir.ActivationFunctionType.Lrelu,
                            bias=b1c[:, 0:1], alpha=0.01,
                        )
                        h1Ts.append(h1T)
                    for j in range(SUPA):
                        b = s * SUPA + j
                        h1T = h1Ts[j // 4]
                        jj = j % 4
                        ps2 = pp2.tile([P, D], dtype=f32, space="PSUM")
                        nc.tensor.matmul(
                            out=ps2[:], lhsT=h1T[:, jj * P:(jj + 1) * P], rhs=w2[:],
                            start=True, stop=False,
                        )
                        nc.tensor.matmul(
                            out=ps2[:], lhsT=ones[:], rhs=b2r[:], start=False, stop=True
                        )
                        nc.vector.tensor_tensor(
                            out=o_super[:, j, :], in0=ps2[:],
                            in1=invs[:, b:b + 1].to_broadcast([P, D]),
                            op=mybir.AluOpType.mult,
                        )
                    nc.gpsimd.dma_start(
                        out=out_ext[:, s * SUPA:(s + 1) * SUPA, :], in_=o_super[:]
                    )
    nc.finalize()
    return nc


def _build_phase_b(ksub, repeat=1):
    bass, bacc, mybir, tile = _ctx()
    f32 = mybir.dt.float32
    bf16 = mybir.dt.bfloat16
    f8 = mybir.dt.float8e4
    KS = SUPB * ksub              # stream columns per supertile
    nc = bacc.Bacc("TRN2", target_bir_lowering=False, debug=False)
    ed_ext = nc.declare_dram_parameter("edB", [NSUP, P, KS, D], f8, isOutput=False)
    lrecv_ext = nc.declare_dram_parameter("lrecvT", [P, NB * ksub], bf16, isOutput=False)
    iota_ext = nc.declare_dram_parameter("iota_rep", [P, P, KS], bf16, isOutput=False)
    wd_ext = nc.declare_dram_parameter("Wd", [D, C], bf16, isOutput=False)
    bd_ext = nc.declare_dram_parameter("bd", [1, C], bf16, isOutput=False)
    sq_ext = nc.declare_dram_parameter("sq_row", [1, NPAD], bf16, isOutput=False)
    prod_ext = nc.declare_dram_parameter("prod_pc", [P, NB], f32, isOutput=False)
    out_ext = nc.declare_dram_parameter("h2s_pm", [P, NB, C], bf16, isOutput=True)

    with tile.TileContext(nc) as tc:
        with (
            tc.tile_pool(name="const", bufs=1) as cp,
            tc.tile_pool(name="gath", bufs=3) as gp,
            tc.tile_pool(name="oneh", bufs=3) as mp,
            tc.tile_pool(name="mid", bufs=6) as sp,
            tc.tile_pool(name="outp", bufs=3) as op,
            tc.tile_pool(name="psuma", bufs=3, space="PSUM") as ppa,
            tc.tile_pool(name="psumo", bufs=3, space="PSUM") as ppo,
        ):
            iota = cp.tile([P, P, KS], dtype=bf16)
            nc.sync.dma_start(out=iota[:], in_=iota_ext[:])
            wd = cp.tile([D, C], dtype=bf16)
            nc.sync.dma_start(out=wd[:], in_=wd_ext[:])
            bd = cp.tile([1, C], dtype=bf16)
            nc.sync.dma_start(out=bd[:], in_=bd_ext[:])
            sq = cp.tile([1, NPAD], dtype=bf16)
            nc.sync.dma_start(out=sq[:], in_=sq_ext[:])
            prod = cp.tile([P, NB], dtype=f32)
            nc.sync.dma_start(out=prod[:], in_=prod_ext[:])
            lrc = cp.tile([P, NB * ksub], dtype=bf16)
            nc.sync.dma_start(out=lrc[:], in_=lrecv_ext[:])

            with _loop(tc, repeat):
                for s in range(NSUP):
                    g = gp.tile([P, KS, D], dtype=f8)
                    nc.sync.dma_start(out=g[:], in_=ed_ext[s, :, :, :])
                    # one-hot in [p, r, k] layout: every operand has a packed
                    # 2-byte last dim, so the DVE 4x perf mode applies
                    m = mp.tile([P, P, KS], dtype=bf16)
                    nc.vector.tensor_tensor(
                        out=m[:],
                        in0=lrc[:, s * KS:(s + 1) * KS].unsqueeze(1)
                            .to_broadcast([P, P, KS]),
                        in1=iota[:],
                        op=mybir.AluOpType.is_equal,
                    )
                    o_super = op.tile([P, SUPB, C], dtype=bf16)
                    for j in range(SUPB):
                        b = s * SUPB + j
                        r0 = b * P
                        ps_aggT = ppa.tile([D, P], dtype=f32, space="PSUM")
                        for k in range(ksub):
                            col = j * ksub + k
                            nc.tensor.matmul(
                                out=ps_aggT[:], lhsT=g[:, col, :],
                                rhs=m[:, :, col],
                                start=(k == 0), stop=(k == ksub - 1),
                            )
                        hT = sp.tile([D, P], dtype=bf16)
                        nc.scalar.activation(
                            out=hT[:], in_=ps_aggT[:],
                            func=mybir.ActivationFunctionType.Lrelu, alpha=0.01,
                        )
                        ps_o = ppo.tile([P, C], dtype=f32, space="PSUM")
                        nc.tensor.matmul(
                            out=ps_o[:], lhsT=hT[:], rhs=wd[:], start=True, stop=False
                        )
                        nc.tensor.matmul(
                            out=ps_o[:], lhsT=sq[0:1, r0:r0 + P], rhs=bd[:],
                            start=False, stop=True,
                        )
                        nc.scalar.activation(
                            out=o_super[:, j, :], in_=ps_o[:],
                            func=mybir.ActivationFunctionType.Copy,
                            scale=prod[:, b:b + 1],
                        )
                    nc.gpsimd.dma_start(
                        out=out_ext[:, s * SUPB:(s + 1) * SUPB, :], in_=o_super[:]
                    )
    nc.finalize()
    return nc


def _build_phase_c(ksub, repeat=1):
    bass, bacc, mybir, tile = _ctx()
    f32 = mybir.dt.float32
    bf16 = mybir.dt.bfloat16
    KS = SUPB * ksub
    nc = bacc.Bacc("TRN2", target_bir_lowering=False, debug=False)
    ed_ext = nc.declare_dram_parameter("edC", [NSUP, P, KS * C], bf16, isOutput=False)
    lrecv_ext = nc.declare_dram_parameter("lrecvT", [P, NB * ksub], bf16, isOutput=False)
    iota_ext = nc.declare_dram_parameter("iota_rep", [P, P, KS], bf16, isOutput=False)
    invr_ext = nc.declare_dram_parameter("invr_pc", [P, NB], f32, isOutput=False)
    out_ext = nc.declare_dram_parameter("res_pm", [P, NB, C], f32, isOutput=True)

    with tile.TileContext(nc) as tc:
        with (
            tc.tile_pool(name="const", bufs=1) as cp,
            tc.tile_pool(name="gath", bufs=3) as gp,
            tc.tile_pool(name="oneh", bufs=3) as mp,
            tc.tile_pool(name="mid", bufs=8) as sp,
            tc.tile_pool(name="outp", bufs=3) as op,
            tc.tile_pool(name="psuma", bufs=4, space="PSUM") as ppa,
        ):
            iota = cp.tile([P, P, KS], dtype=bf16)
            nc.sync.dma_start(out=iota[:], in_=iota_ext[:])
            invr = cp.tile([P, NB], dtype=f32)
            nc.sync.dma_start(out=invr[:], in_=invr_ext[:])
            lrc = cp.tile([P, NB * ksub], dtype=bf16)
            nc.sync.dma_start(out=lrc[:], in_=lrecv_ext[:])

            with _loop(tc, repeat):
                for s in range(NSUP):
                    g = gp.tile([P, KS * C], dtype=bf16)
                    nc.sync.dma_start(out=g[:], in_=ed_ext[s, :, :])
                    m = mp.tile([P, P, KS], dtype=bf16)
                    nc.vector.tensor_tensor(
                        out=m[:],
                        in0=lrc[:, s * KS:(s + 1) * KS].unsqueeze(1)
                            .to_broadcast([P, P, KS]),
                        in1=iota[:],
                        op=mybir.AluOpType.is_equal,
                    )
                    o_super = op.tile([P, SUPB, C], dtype=f32)
                    for j in range(SUPB):
                        b = s * SUPB + j
                        ps = ppa.tile([P, C], dtype=f32, space="PSUM")
                        for k in range(ksub):
                            col = j * ksub + k
                            nc.tensor.matmul(
                                out=ps[:], lhsT=m[:, :, col],
                                rhs=g[:, col * C:(col + 1) * C],
                                start=(k == 0), stop=(k == ksub - 1),
                            )
                        # logits are bounded well below exp-overflow range, so
                        # no max-subtraction; denominator via accum_out and
                        # the divide on the (otherwise idle) Pool engine.
                        ex = sp.tile([P, C], dtype=f32)
                        den = sp.tile([P, 1], dtype=f32)
                        nc.scalar.activation(
                            out=ex[:], in_=ps[:],
                            func=mybir.ActivationFunctionType.Exp,
                            scale=invr[:, b:b + 1],
                            accum_out=den[:, 0:1],
                        )
                        rec = sp.tile([P, 1], dtype=f32)
                        nc.vector.reciprocal(rec[:], den[:])
                        nc.scalar.activation(
                            out=o_super[:, j, :], in_=ex[:],
                            func=mybir.ActivationFunctionType.Copy,
                            scale=rec[:, 0:1],
                        )
                    nc.scalar.dma_start(
                        out=out_ext[:, s * SUPB:(s + 1) * SUPB, :], in_=o_super[:]
                    )
    nc.finalize()
    return nc


_EXEC_TIMES = []
_LAST = {}


def _run(nc, in_maps):
    from concourse.bass_utils import run_bass_kernel_spmd
    res = run_bass_kernel_spmd(nc, in_maps, core_ids=list(range(NCORES)))
    if res.exec_time_ns is not None:
        _EXEC_TIMES.append(res.exec_time_ns)
    return res.results


def _prep(x, senders, receivers, W1, b1, W2, b2, Wd, bd):
    """Host-side index preprocessing and per-core input assembly."""
    deg_s = np.bincount(senders, minlength=N).astype(np.float32)
    deg_r = np.bincount(receivers, minlength=N).astype(np.float32)
    inv_s = (1.0 / np.sqrt(np.maximum(deg_s, 1.0))).astype(np.float32)
    inv_r = (1.0 / np.sqrt(np.maximum(deg_r, 1.0))).astype(np.float32)
    sq_r = np.sqrt(np.maximum(deg_r, 1.0)).astype(np.float32)

    order = np.argsort(receivers, kind="stable")
    rs = receivers[order]
    ss = senders[order]

    bounds = []
    for c in range(NCORES):
        for b in range(NB):
            lo = c * NS + b * P
            hi = c * NS + min((b + 1) * P, NS)
            bounds.append((lo, hi))
    lows = np.searchsorted(rs, [lo for lo, _ in bounds], side="left")
    highs = np.searchsorted(rs, [hi for _, hi in bounds], side="left")
    counts = highs - lows
    kmax = int(counts.max())
    ksub = max(1, (kmax + P - 1) // P)
    K = ksub * P

    # slot tables: eidx[c, b, p, k] = global edge (into ss) or -1 for pad
    eidx = np.full((NCORES, NB, P, ksub), -1, np.int64)
    lrecvT = np.full((NCORES, NB, P, ksub), -1.0, np.float32)
    lbuf = np.empty(K, np.float32)
    ebuf = np.empty(K, np.int64)
    for c in range(NCORES):
        for b in range(NB):
            i = c * NB + b
            lo, hi, m = lows[i], highs[i], counts[i]
            ebuf[:] = -1
            lbuf[:] = -1.0
            ebuf[:m] = np.arange(lo, hi)
            lbuf[:m] = (rs[lo:hi] - (c * NS + b * P)).astype(np.float32)
            eidx[c, b] = ebuf.reshape(ksub, P).T
            lrecvT[c, b] = lbuf.reshape(ksub, P).T
    # sender per slot (pad -> sender 0, killed by -1 lrecv one-hot)
    send_slot = np.where(eidx >= 0, ss[np.clip(eidx, 0, E - 1)], 0)  # [NC,NB,P,ksub]
    # device lrecv layout [P, NB*ksub]: col b*ksub+k
    lrecv_dev = np.ascontiguousarray(
        lrecvT.transpose(0, 2, 1, 3).reshape(NCORES, P, NB * ksub)
    ).astype(BF16)
    # stream slot layout [NSUP, P, SUPB*ksub] of sender ids, for host gather
    send_stream = np.ascontiguousarray(
        send_slot.reshape(NCORES, NSUP, SUPB, P, ksub)
        .transpose(0, 1, 3, 2, 4)
        .reshape(NCORES, NSUP, P, SUPB * ksub)
    )

    KS = SUPB * ksub
    iota_rep = np.ascontiguousarray(np.broadcast_to(
        np.arange(P, dtype=np.float32)[None, :, None], (P, P, KS))).astype(BF16)
    ones_row = np.ones((1, P), np.float32).astype(BF16)

    def pcol(v, fill=1.0):  # [N] -> [NCORES, P, NB]
        out = np.full((NCORES, NPAD), fill, np.float32)
        out[:, :NS] = v.reshape(NCORES, NS)
        return np.ascontiguousarray(out.reshape(NCORES, NB, P).transpose(0, 2, 1))

    invs_pc = pcol(inv_s)
    invr_pc = pcol(inv_r)
    ninvr_pc = pcol(-inv_r, fill=-1.0)
    prod_pc = pcol(inv_r * inv_s)
    sq_row = np.ones((NCORES, 1, NPAD), np.float32)
    sq_row[:, 0, :NS] = sq_r.reshape(NCORES, NS)
    sq_row = sq_row.astype(BF16)

    xT = np.zeros((NCORES, D, NPAD), np.float32)
    for c in range(NCORES):
        xT[c, :, :NS] = x[c * NS:(c + 1) * NS].T
    xT = xT.astype(BF16)

    return dict(
        ksub=ksub, xT=xT, send_stream=send_stream, lrecv_dev=lrecv_dev,
        iota_rep=iota_rep, ones_row=ones_row, invs_pc=invs_pc, invr_pc=invr_pc,
        prod_pc=prod_pc, sq_row=sq_row,
        W1=W1.astype(BF16), b1c=b1.reshape(D, 1).astype(np.float32),
        W2=W2.astype(BF16), b2r=b2.reshape(1, D).astype(BF16),
        Wd=Wd.astype(BF16), bd=bd.reshape(1, C).astype(BF16),
    )


def _pm_to_nodes(pm):
    """[NCORES][P, NB, W] core outputs -> [N, W] node-major."""
    w = pm[0].shape[-1]
    out = np.empty((NCORES, NS, w), pm[0].dtype)
    for c in range(NCORES):
        out[c] = np.ascontiguousarray(pm[c].transpose(1, 0, 2)).reshape(NPAD, w)[:NS]
    return out.reshape(N, w)


def kernel(x, senders, receivers, W1, b1, W2, b2, Wd, bd):
    x = np.asarray(x, np.float32)
    senders = np.asarray(senders, np.int32)
    receivers = np.asarray(receivers, np.int32)
    pr = _prep(x, senders, receivers,
               np.asarray(W1, np.float32), np.asarray(b1, np.float32),
               np.asarray(W2, np.float32), np.asarray(b2, np.float32),
               np.asarray(Wd, np.float32), np.asarray(bd, np.float32))
    ksub = pr["ksub"]
    KS = SUPB * ksub

    # ---- phase A ----
    nc_a = _build_phase_a()
    maps_a = [
        {"xT": pr["xT"][c], "W1": pr["W1"], "b1c": pr["b1c"], "W2": pr["W2"],
         "b2r": pr["b2r"], "ones_row": pr["ones_row"], "invs_pc": pr["invs_pc"][c]}
        for c in range(NCORES)
    ]
    _LAST["A"] = maps_a
    res_a = _run(nc_a, maps_a)
    h1s = _pm_to_nodes([np.asarray(r["h1s_pm"]) for r in res_a])  # [N, D] fp8

    # ---- host halo exchange: edge-row streams for phase B ----
    sstr = pr["send_stream"]  # [NC, NSUP, P, KS]
    edB = [np.ascontiguousarray(
        h1s[sstr[c].reshape(-1)].reshape(NSUP, P, KS, D)) for c in range(NCORES)]

    nc_b = _build_phase_b(ksub)
    maps_b = [
        {"edB": edB[c], "lrecvT": pr["lrecv_dev"][c], "iota_rep": pr["iota_rep"],
         "Wd": pr["Wd"], "bd": pr["bd"], "sq_row": pr["sq_row"][c],
         "prod_pc": pr["prod_pc"][c]}
        for c in range(NCORES)
    ]
    _LAST["B"] = maps_b
    res_b = _run(nc_b, maps_b)
    h2s = _pm_to_nodes([np.asarray(r["h2s_pm"]) for r in res_b])  # [N, C] bf16

    # ---- host halo exchange for phase C ----
    edC = [np.ascontiguousarray(
        h2s[sstr[c].reshape(-1)].reshape(NSUP, P, KS * C)) for c in range(NCORES)]

    nc_c = _build_phase_c(ksub)
    maps_c = [
        {"edC": edC[c], "lrecvT": pr["lrecv_dev"][c], "iota_rep": pr["iota_rep"],
         "invr_pc": pr["invr_pc"][c]}
        for c in range(NCORES)
    ]
    _LAST["C"] = maps_c
    res_c = _run(nc_c, maps_c)
    out = _pm_to_nodes([np.asarray(r["res_pm"]) for r in res_c])  # [N, C] f32
    _LAST["ksub"] = ksub
    return np.ascontiguousarray(out).astype(np.float32)


# revision 31
# speedup vs baseline: 1.4537x; 1.4537x over previous
"""GCN (2-layer graph convolution, symmetric norm) on 8 TRN2 NeuronCores.

Node-sharded graph/data-parallel, 3 launches (per sharding hint):
 - Phase A (node-sharded MLP, transposed dataflow): each core computes
   h1s = (lrelu(x@W1+b1)@W2+b2) * rsqrt(max(deg_s,1)) for its 12500-node
   range (fp8e4m3 table out). Host supplies x pre-transposed, so zero
   on-device transposes:  h1T = W1^T @ xT (lhsT=W1), Lrelu+bias on the
   scalar engine, then h2 = h1T^T @ W2 (lhsT=h1T) lands row-major.
 - Halo exchange between launches is host-mediated: the host gathers
   h1s[senders] / h2s[senders] into per-core, receiver-sorted edge-row
   streams (partition-major layout), so each launch only does full-
   bandwidth sequential DMA - no on-device random access.
 - Phase B (edge-sharded by receiver): per 128-receiver block, segment-sum
   the streamed fp8 rows with one-hot matmuls accumulating the TRANSPOSED
   aggregate (lhsT=g, rhs=onehot), so the Lrelu'd result feeds the decode
   matmul directly as lhsT with no transpose. inv_r/bias/inv_s algebra is
   folded into a rank-1 bias matmul (sqrt(deg_r) x bd) and one per-
   partition scale (inv_r*inv_s). One-hot matrices for 49 blocks x ksub
   are built in ONE DVE op per supertile via 3D broadcast is_equal.
 - Phase C: same aggregation over bf16 h2s edge rows (40-wide), then a
   fused softmax: Exp activation with scale=inv_r, bias=-inv_r*max,
   accum_out for the denominator.
"""

import numpy as np
import ml_dtypes

N = 100000
E = 600000
D = 128
C = 40
NCORES = 8
NS = N // NCORES          # 12500 nodes per core
P = 128
NB = (NS + P - 1) // P    # 98 blocks per core
NPAD = NB * P             # 12544
SUPA = 14                 # phase-A supertile (blocks per DMA); NB % SUPA == 0
SUPB = 7                  # phase-B/C supertile (blocks per stream tile)
NSUP = NB // SUPB         # 14

BF16 = ml_dtypes.bfloat16
F8 = ml_dtypes.float8_e4m3


def _ctx():
    from concourse import bass, bacc, mybir, tile
    return bass, bacc, mybir, tile


def _loop(tc, repeat):
    import contextlib
    if repeat > 1:
        return tc.For_i(0, repeat)
    return contextlib.nullcontext()


def _build_phase_a(repeat=1):
    bass, bacc, mybir, tile = _ctx()
    f32 = mybir.dt.float32
    bf16 = mybir.dt.bfloat16
    f8 = mybir.dt.float8e4
    nc = bacc.Bacc("TRN2", target_bir_lowering=False, debug=False)
    xT_ext = nc.declare_dram_parameter("xT", [D, NPAD], bf16, isOutput=False)
    w1_ext = nc.declare_dram_parameter("W1", [D, D], bf16, isOutput=False)
    b1_ext = nc.declare_dram_parameter("b1c", [D, 1], f32, isOutput=False)
    w2_ext = nc.declare_dram_parameter("W2", [D, D], bf16, isOutput=False)
    b2_ext = nc.declare_dram_parameter("b2r", [1, D], bf16, isOutput=False)
    ones_ext = nc.declare_dram_parameter("ones_row", [1, P], bf16, isOutput=False)
    invs_ext = nc.declare_dram_parameter("invs_pc", [P, NB], f32, isOutput=False)
    out_ext = nc.declare_dram_parameter("h1s_pm", [P, NB, D], f8, isOutput=True)

    with tile.TileContext(nc) as tc:
        with (
            tc.tile_pool(name="const", bufs=1) as cp,
            tc.tile_pool(name="xin", bufs=3) as xp,
            tc.tile_pool(name="mid", bufs=6) as sp,
            tc.tile_pool(name="outp", bufs=3) as op,
            tc.tile_pool(name="psum1", bufs=3, space="PSUM") as pp1,
            tc.tile_pool(name="psum2", bufs=3, space="PSUM") as pp2,
        ):
            w1 = cp.tile([D, D], dtype=bf16)
            nc.sync.dma_start(out=w1[:], in_=w1_ext[:])
            b1c = cp.tile([D, 1], dtype=f32)
            nc.sync.dma_start(out=b1c[:], in_=b1_ext[:])
            w2 = cp.tile([D, D], dtype=bf16)
            nc.sync.dma_start(out=w2[:], in_=w2_ext[:])
            b2r = cp.tile([1, D], dtype=bf16)
            nc.sync.dma_start(out=b2r[:], in_=b2_ext[:])
            ones = cp.tile([1, P], dtype=bf16)
            nc.sync.dma_start(out=ones[:], in_=ones_ext[:])
            invs = cp.tile([P, NB], dtype=f32)
            nc.sync.dma_start(out=invs[:], in_=invs_ext[:])

            with _loop(tc, repeat):
                for s in range(NB // SUPA):
                    xt = xp.tile([D, SUPA * P], dtype=bf16)
                    nc.sync.dma_start(
                        out=xt[:], in_=xT_ext[:, s * SUPA * P:(s + 1) * SUPA * P]
                    )
                    o_super = op.tile([P, SUPA, D], dtype=f8)
                    # mm1 + lrelu batched over groups of 4 blocks (one PSUM bank)
                    h1Ts = []
                    for g0 in range(0, SUPA, 4):
                        gw = min(4, SUPA - g0)
                        ps1 = pp1.tile([D, 4 * P], dtype=f32, space="PSUM")
                        nc.tensor.matmul(
                            out=ps1[:, :gw * P], lhsT=w1[:],
                            rhs=xt[:, g0 * P:(g0 + gw) * P],
                            start=True, stop=True,
                        )
                        h1T = sp.tile([D, 4 * P], dtype=bf16)
                        nc.scalar.activation(
                            out=h1T[:, :gw * P], in_=ps1[:, :gw * P],
                            func=mybir.ActivationFunctionType.Lrelu,
                            bias=b1c[:, 0:1], alpha=0.01,
                        )
                        h1Ts.append(h1T)
                    for j in range(SUPA):
                        b = s * SUPA + j
                        h1T = h1Ts[j // 4]
                        jj = j % 4
                        ps2 = pp2.tile([P, D], dtype=f32, space="PSUM")
                        nc.tensor.matmul(
                            out=ps2[:], lhsT=h1T[:, jj * P:(jj + 1) * P], rhs=w2[:],
                            start=True, stop=False,
                        )
                        nc.tensor.matmul(
                            out=ps2[:], lhsT=ones[:], rhs=b2r[:], start=False, stop=True
                        )
                        nc.vector.tensor_tensor(
                            out=o_super[:, j, :], in0=ps2[:],
                            in1=invs[:, b:b + 1].to_broadcast([P, D]),
                            op=mybir.AluOpType.mult,
                        )
                    nc.gpsimd.dma_start(
                        out=out_ext[:, s * SUPA:(s + 1) * SUPA, :], in_=o_super[:]
                    )
    nc.finalize()
    return nc


def _build_phase_b(ksub, repeat=1):
    bass, bacc, mybir, tile = _ctx()
    f32 = mybir.dt.float32
    bf16 = mybir.dt.bfloat16
    f8 = mybir.dt.float8e4
    KS = SUPB * ksub              # stream columns per supertile
    nc = bacc.Bacc("TRN2", target_bir_lowering=False, debug=False)
    ed_ext = nc.declare_dram_parameter("edB", [NSUP, P, KS, D], f8, isOutput=False)
    lrecv_ext = nc.declare_dram_parameter("lrecvT", [P, NB * ksub], bf16, isOutput=False)
    iota_ext = nc.declare_dram_parameter("iota", [P, P], bf16, isOutput=False)
    wd_ext = nc.declare_dram_parameter("Wd", [D, C], bf16, isOutput=False)
    bd_ext = nc.declare_dram_parameter("bd", [1, C], bf16, isOutput=False)
    sq_ext = nc.declare_dram_parameter("sq_row", [1, NPAD], bf16, isOutput=False)
    prod_ext = nc.declare_dram_parameter("prod_pc", [P, NB], f32, isOutput=False)
    out_ext = nc.declare_dram_parameter("h2s_pm", [P, NB, C], bf16, isOutput=True)

    with tile.TileContext(nc) as tc:
        with (
            tc.tile_pool(name="const", bufs=1) as cp,
            tc.tile_pool(name="gath", bufs=3) as gp,
            tc.tile_pool(name="oneh", bufs=3) as mp,
            tc.tile_pool(name="mid", bufs=6) as sp,
            tc.tile_pool(name="outp", bufs=3) as op,
            tc.tile_pool(name="psuma", bufs=3, space="PSUM") as ppa,
            tc.tile_pool(name="psumo", bufs=3, space="PSUM") as ppo,
        ):
            iota = cp.tile([P, P], dtype=bf16)
            nc.sync.dma_start(out=iota[:], in_=iota_ext[:])
            wd = cp.tile([D, C], dtype=bf16)
            nc.sync.dma_start(out=wd[:], in_=wd_ext[:])
            bd = cp.tile([1, C], dtype=bf16)
            nc.sync.dma_start(out=bd[:], in_=bd_ext[:])
            sq = cp.tile([1, NPAD], dtype=bf16)
            nc.sync.dma_start(out=sq[:], in_=sq_ext[:])
            prod = cp.tile([P, NB], dtype=f32)
            nc.sync.dma_start(out=prod[:], in_=prod_ext[:])
            lrc = cp.tile([P, NB * ksub], dtype=bf16)
            nc.sync.dma_start(out=lrc[:], in_=lrecv_ext[:])

            with _loop(tc, repeat):
                for s in range(NSUP):
                    g = gp.tile([P, KS, D], dtype=f8)
                    nc.sync.dma_start(out=g[:], in_=ed_ext[s, :, :, :])
                    # one-hot in [p, k, r] layout: contiguous r-slices so the
                    # matmul's MOVING operand is unit-stride (strided moving
                    # operands are slow on HW; strided stationary is fine)
                    m = mp.tile([P, KS, P], dtype=bf16)
                    nc.vector.tensor_tensor(
                        out=m[:],
                        in0=lrc[:, s * KS:(s + 1) * KS].unsqueeze(2)
                            .to_broadcast([P, KS, P]),
                        in1=iota[:].unsqueeze(1).to_broadcast([P, KS, P]),
                        op=mybir.AluOpType.is_equal,
                    )
                    o_super = op.tile([P, SUPB, C], dtype=bf16)
                    for j in range(SUPB):
                        b = s * SUPB + j
                        r0 = b * P
                        ps_aggT = ppa.tile([D, P], dtype=f32, space="PSUM")
                        for k in range(ksub):
                            col = j * ksub + k
                            nc.tensor.matmul(
                                out=ps_aggT[:], lhsT=g[:, col, :],
                                rhs=m[:, col, :],
                                start=(k == 0), stop=(k == ksub - 1),
                            )
                        hT = sp.tile([D, P], dtype=bf16)
                        nc.scalar.activation(
                            out=hT[:], in_=ps_aggT[:],
                            func=mybir.ActivationFunctionType.Lrelu, alpha=0.01,
                        )
                        ps_o = ppo.tile([P, C], dtype=f32, space="PSUM")
                        nc.tensor.matmul(
                            out=ps_o[:], lhsT=hT[:], rhs=wd[:], start=True, stop=False
                        )
                        nc.tensor.matmul(
                            out=ps_o[:], lhsT=sq[0:1, r0:r0 + P], rhs=bd[:],
                            start=False, stop=True,
                        )
                        nc.scalar.activation(
                            out=o_super[:, j, :], in_=ps_o[:],
                            func=mybir.ActivationFunctionType.Copy,
                            scale=prod[:, b:b + 1],
                        )
                    nc.gpsimd.dma_start(
                        out=out_ext[:, s * SUPB:(s + 1) * SUPB, :], in_=o_super[:]
                    )
    nc.finalize()
    return nc


def _build_phase_c(ksub, repeat=1):
    bass, bacc, mybir, tile = _ctx()
    f32 = mybir.dt.float32
    bf16 = mybir.dt.bfloat16
    KS = SUPB * ksub
    nc = bacc.Bacc("TRN2", target_bir_lowering=False, debug=False)
    ed_ext = nc.declare_dram_parameter("edC", [NSUP, P, KS * C], bf16, isOutput=False)
    lrecv_ext = nc.declare_dram_parameter("lrecvT", [P, NB * ksub], bf16, isOutput=False)
    iota_ext = nc.declare_dram_parameter("iota_rep", [P, P, KS], bf16, isOutput=False)
    invr_ext = nc.declare_dram_parameter("invr_pc", [P, NB], f32, isOutput=False)
    out_ext = nc.declare_dram_parameter("res_pm", [P, NB, C], f32, isOutput=True)

    with tile.TileContext(nc) as tc:
        with (
            tc.tile_pool(name="const", bufs=1) as cp,
            tc.tile_pool(name="gath", bufs=3) as gp,
            tc.tile_pool(name="oneh", bufs=3) as mp,
            tc.tile_pool(name="mid", bufs=8) as sp,
            tc.tile_pool(name="outp", bufs=3) as op,
            tc.tile_pool(name="psuma", bufs=4, space="PSUM") as ppa,
        ):
            iota = cp.tile([P, P, KS], dtype=bf16)
            nc.sync.dma_start(out=iota[:], in_=iota_ext[:])
            invr = cp.tile([P, NB], dtype=f32)
            nc.sync.dma_start(out=invr[:], in_=invr_ext[:])
            lrc = cp.tile([P, NB * ksub], dtype=bf16)
            nc.sync.dma_start(out=lrc[:], in_=lrecv_ext[:])

            with _loop(tc, repeat):
                for s in range(NSUP):
                    g = gp.tile([P, KS * C], dtype=bf16)
                    nc.sync.dma_start(out=g[:], in_=ed_ext[s, :, :])
                    m = mp.tile([P, P, KS], dtype=bf16)
                    nc.vector.tensor_tensor(
                        out=m[:],
                        in0=lrc[:, s * KS:(s + 1) * KS].unsqueeze(1)
                            .to_broadcast([P, P, KS]),
                        in1=iota[:],
                        op=mybir.AluOpType.is_equal,
                    )
                    o_super = op.tile([P, SUPB, C], dtype=f32)
                    for j in range(SUPB):
                        b = s * SUPB + j
                        ps = ppa.tile([P, C], dtype=f32, space="PSUM")
                        for k in range(ksub):
                            col = j * ksub + k
                            nc.tensor.matmul(
                                out=ps[:], lhsT=m[:, :, col],
                                rhs=g[:, col * C:(col + 1) * C],
                                start=(k == 0), stop=(k == ksub - 1),
                            )
                        # logits are bounded well below exp-overflow range, so
                        # no max-subtraction; denominator via accum_out and
                        # the divide on the (otherwise idle) Pool engine.
                        ex = sp.tile([P, C], dtype=f32)
                        den = sp.tile([P, 1], dtype=f32)
                        nc.scalar.activation(
                            out=ex[:], in_=ps[:],
                            func=mybir.ActivationFunctionType.Exp,
                            scale=invr[:, b:b + 1],
                            accum_out=den[:, 0:1],
                        )
                        rec = sp.tile([P, 1], dtype=f32)
                        nc.vector.reciprocal(rec[:], den[:])
                        nc.scalar.activation(
                            out=o_super[:, j, :], in_=ex[:],
                            func=mybir.ActivationFunctionType.Copy,
                            scale=rec[:, 0:1],
                        )
                    nc.scalar.dma_start(
                        out=out_ext[:, s * SUPB:(s + 1) * SUPB, :], in_=o_super[:]
                    )
    nc.finalize()
    return nc


_EXEC_TIMES = []
_LAST = {}


def _run(nc, in_maps):
    from concourse.bass_utils import run_bass_kernel_spmd
    res = run_bass_kernel_spmd(nc, in_maps, core_ids=list(range(NCORES)))
    if res.exec_time_ns is not None:
        _EXEC_TIMES.append(res.exec_time_ns)
    return res.results


def _prep(x, senders, receivers, W1, b1, W2, b2, Wd, bd):
    """Host-side index preprocessing and per-core input assembly."""
    deg_s = np.bincount(senders, minlength=N).astype(np.float32)
    deg_r = np.bincount(receivers, minlength=N).astype(np.float32)
    inv_s = (1.0 / np.sqrt(np.maximum(deg_s, 1.0))).astype(np.float32)
    inv_r = (1.0 / np.sqrt(np.maximum(deg_r, 1.0))).astype(np.float32)
    sq_r = np.sqrt(np.maximum(deg_r, 1.0)).astype(np.float32)

    order = np.argsort(receivers, kind="stable")
    rs = receivers[order]
    ss = senders[order]

    bounds = []
    for c in range(NCORES):
        for b in range(NB):
            lo = c * NS + b * P
            hi = c * NS + min((b + 1) * P, NS)
            bounds.append((lo, hi))
    lows = np.searchsorted(rs, [lo for lo, _ in bounds], side="left")
    highs = np.searchsorted(rs, [hi for _, hi in bounds], side="left")
    counts = highs - lows
    kmax = int(counts.max())
    ksub = max(1, (kmax + P - 1) // P)
    K = ksub * P

    # slot tables: eidx[c, b, p, k] = global edge (into ss) or -1 for pad
    eidx = np.full((NCORES, NB, P, ksub), -1, np.int64)
    lrecvT = np.full((NCORES, NB, P, ksub), -1.0, np.float32)
    lbuf = np.empty(K, np.float32)
    ebuf = np.empty(K, np.int64)
    for c in range(NCORES):
        for b in range(NB):
            i = c * NB + b
            lo, hi, m = lows[i], highs[i], counts[i]
            ebuf[:] = -1
            lbuf[:] = -1.0
            ebuf[:m] = np.arange(lo, hi)
            lbuf[:m] = (rs[lo:hi] - (c * NS + b * P)).astype(np.float32)
            eidx[c, b] = ebuf.reshape(ksub, P).T
            lrecvT[c, b] = lbuf.reshape(ksub, P).T
    # sender per slot (pad -> sender 0, killed by -1 lrecv one-hot)
    send_slot = np.where(eidx >= 0, ss[np.clip(eidx, 0, E - 1)], 0)  # [NC,NB,P,ksub]
    # device lrecv layout [P, NB*ksub]: col b*ksub+k
    lrecv_dev = np.ascontiguousarray(
        lrecvT.transpose(0, 2, 1, 3).reshape(NCORES, P, NB * ksub)
    ).astype(BF16)
    # stream slot layout [NSUP, P, SUPB*ksub] of sender ids, for host gather
    send_stream = np.ascontiguousarray(
        send_slot.reshape(NCORES, NSUP, SUPB, P, ksub)
        .transpose(0, 1, 3, 2, 4)
        .reshape(NCORES, NSUP, P, SUPB * ksub)
    )

    KS = SUPB * ksub
    iota = np.tile(np.arange(P, dtype=np.float32)[None, :], (P, 1)).astype(BF16)
    iota_rep = np.ascontiguousarray(np.broadcast_to(
        np.arange(P, dtype=np.float32)[None, :, None], (P, P, KS))).astype(BF16)
    ones_row = np.ones((1, P), np.float32).astype(BF16)

    def pcol(v, fill=1.0):  # [N] -> [NCORES, P, NB]
        out = np.full((NCORES, NPAD), fill, np.float32)
        out[:, :NS] = v.reshape(NCORES, NS)
        return np.ascontiguousarray(out.reshape(NCORES, NB, P).transpose(0, 2, 1))

    invs_pc = pcol(inv_s)
    invr_pc = pcol(inv_r)
    ninvr_pc = pcol(-inv_r, fill=-1.0)
    prod_pc = pcol(inv_r * inv_s)
    sq_row = np.ones((NCORES, 1, NPAD), np.float32)
    sq_row[:, 0, :NS] = sq_r.reshape(NCORES, NS)
    sq_row = sq_row.astype(BF16)

    xT = np.zeros((NCORES, D, NPAD), np.float32)
    for c in range(NCORES):
        xT[c, :, :NS] = x[c * NS:(c + 1) * NS].T
    xT = xT.astype(BF16)

    return dict(
        ksub=ksub, xT=xT, send_stream=send_stream, lrecv_dev=lrecv_dev,
        iota=iota, iota_rep=iota_rep, ones_row=ones_row,
        invs_pc=invs_pc, invr_pc=invr_pc,
        prod_pc=prod_pc, sq_row=sq_row,
        W1=W1.astype(BF16), b1c=b1.reshape(D, 1).astype(np.float32),
        W2=W2.astype(BF16), b2r=b2.reshape(1, D).astype(BF16),
        Wd=Wd.astype(BF16), bd=bd.reshape(1, C).astype(BF16),
    )


def _pm_to_nodes(pm):
    """[NCORES][P, NB, W] core outputs -> [N, W] node-major."""
    w = pm[0].shape[-1]
    out = np.empty((NCORES, NS, w), pm[0].dtype)
    for c in range(NCORES):
        out[c] = np.ascontiguousarray(pm[c].transpose(1, 0, 2)).reshape(NPAD, w)[:NS]
    return out.reshape(N, w)


def kernel(x, senders, receivers, W1, b1, W2, b2, Wd, bd):
    x = np.asarray(x, np.float32)
    senders = np.asarray(senders, np.int32)
    receivers = np.asarray(receivers, np.int32)
    pr = _prep(x, senders, receivers,
               np.asarray(W1, np.float32), np.asarray(b1, np.float32),
               np.asarray(W2, np.float32), np.asarray(b2, np.float32),
               np.asarray(Wd, np.float32), np.asarray(bd, np.float32))
    ksub = pr["ksub"]
    KS = SUPB * ksub

    # ---- phase A ----
    nc_a = _build_phase_a()
    maps_a = [
        {"xT": pr["xT"][c], "W1": pr["W1"], "b1c": pr["b1c"], "W2": pr["W2"],
         "b2r": pr["b2r"], "ones_row": pr["ones_row"], "invs_pc": pr["invs_pc"][c]}
        for c in range(NCORES)
    ]
    _LAST["A"] = maps_a
    res_a = _run(nc_a, maps_a)
    h1s = _pm_to_nodes([np.asarray(r["h1s_pm"]) for r in res_a])  # [N, D] fp8

    # ---- host halo exchange: edge-row streams for phase B ----
    sstr = pr["send_stream"]  # [NC, NSUP, P, KS]
    edB = [np.ascontiguousarray(
        h1s[sstr[c].reshape(-1)].reshape(NSUP, P, KS, D)) for c in range(NCORES)]

    nc_b = _build_phase_b(ksub)
    maps_b = [
        {"edB": edB[c], "lrecvT": pr["lrecv_dev"][c], "iota": pr["iota"],
         "Wd": pr["Wd"], "bd": pr["bd"], "sq_row": pr["sq_row"][c],
         "prod_pc": pr["prod_pc"][c]}
        for c in range(NCORES)
    ]
    _LAST["B"] = maps_b
    res_b = _run(nc_b, maps_b)
    h2s = _pm_to_nodes([np.asarray(r["h2s_pm"]) for r in res_b])  # [N, C] bf16

    # ---- host halo exchange for phase C ----
    edC = [np.ascontiguousarray(
        h2s[sstr[c].reshape(-1)].reshape(NSUP, P, KS * C)) for c in range(NCORES)]

    nc_c = _build_phase_c(ksub)
    maps_c = [
        {"edC": edC[c], "lrecvT": pr["lrecv_dev"][c], "iota_rep": pr["iota_rep"],
         "invr_pc": pr["invr_pc"][c]}
        for c in range(NCORES)
    ]
    _LAST["C"] = maps_c
    res_c = _run(nc_c, maps_c)
    out = _pm_to_nodes([np.asarray(r["res_pm"]) for r in res_c])  # [N, C] f32
    _LAST["ksub"] = ksub
    return np.ascontiguousarray(out).astype(np.float32)
